# revision 4
# baseline (speedup 1.0000x reference)
"""Causal single-head attention (B=4, S=2048, d=1024) on 8 TRN2 NeuronCores.

Sharding (uniform single program, data-parallel batch x interleaved query
blocks):
  core c -> batch b = c//2, subset s = c%2.
  Per batch, the 16 query blocks of 128 rows are split into quads
  t=0..3; core (b,s) owns blocks {4t+2s, 4t+2s+1}. Every core therefore
  runs the identical instruction stream (padded causal limit (t+1)*512
  per quad); the true causal boundary is applied via per-core 0/1 mask
  tiles supplied as input data.

Device algorithm per core (all matmuls float32r, fp32 accumulate):
  P0: qT = Wq @ xq^T (scaled by 1/sqrt(d)) -> DRAM scratch
  P1: kT = Wk @ x^T  (resident SBUF, [d_out, 2048])
  P2: v  = (Wv @ x^T)^T (resident SBUF, [2048, d_out])
  P3: per quad t: scoresT[k,q] = kT^T-slices @ qT-group (transposed
      scores: keys on partitions, queries on free dim), p = exp(scoresT)
      (no max subtraction: |scores| <= ~2), mask, then
      out[q,:] = (pT^T @ v) / (pT^T @ ones)  -- row sums via ones-matmul.
"""
import sys

sys.path.insert(0, "/opt/trn_rl_repo")

import numpy as np

import concourse.bass as bass  # noqa: F401  (bass types used indirectly)
import concourse.mybir as mybir
import concourse.tile as tile
from concourse import bacc
from concourse.bass_utils import run_bass_kernel_spmd

B, S, D = 4, 2048, 1024
DC = D // 128          # 8 contraction chunks
NKB = S // 128         # 16 key blocks
SCALE = 1.0 / float(np.sqrt(D))
F32 = mybir.dt.float32
F32R = mybir.dt.float32r
EXP = mybir.ActivationFunctionType.Exp

_cache = {}


def _r(ap):
    return ap.bitcast(F32R)


def build_nc():
    nc = bacc.Bacc("TRN2", target_bir_lowering=False, debug=False)
    xT = nc.dram_tensor("xT", [D, S], F32R, kind="ExternalInput")
    xTq = nc.dram_tensor("xTq", [D, 1024], F32R, kind="ExternalInput")
    WqT = nc.dram_tensor("WqT", [D, D], F32R, kind="ExternalInput")
    WkT = nc.dram_tensor("WkT", [D, D], F32R, kind="ExternalInput")
    WvT = nc.dram_tensor("WvT", [D, D], F32R, kind="ExternalInput")
    masks = nc.dram_tensor("masks", [4, 128, 256], F32R, kind="ExternalInput")
    out = nc.dram_tensor("out", [1024, D], F32, kind="ExternalOutput")
    qTs = nc.dram_tensor("qTs", [D, 1024], F32R)  # internal scratch

    xT_r = xT[:].rearrange("(dc p) s -> p dc s", p=128)
    xTq_r = xTq[:].rearrange("(dc p) s -> p dc s", p=128)
    WqT_r = WqT[:].rearrange("(dc p) o -> p dc o", p=128)
    WkT_r = WkT[:].rearrange("(dc p) o -> p dc o", p=128)
    WvT_r = WvT[:].rearrange("(dc p) o -> p dc o", p=128)
    qTs_r = qTs[:].rearrange("(oc p) q -> p oc q", p=128)

    with tile.TileContext(nc) as tc:
        # ---------------- P0: Q projection -> qTs scratch ----------------
        with (
            tc.tile_pool(name="p0w", bufs=1) as p0w,
            tc.tile_pool(name="p0ev", bufs=4) as p0ev,
            tc.tile_pool(name="ps0", bufs=4, space="PSUM") as ps0,
        ):
            wq = p0w.tile([128, DC, D], F32R)
            xq = p0w.tile([128, DC, 1024], F32R)
            nc.sync.dma_start(out=wq, in_=WqT_r)
            nc.sync.dma_start(out=xq, in_=xTq_r)
            for oc in range(8):
                pss = [ps0.tile([128, 512], F32, tag="ps0", name=f"ps0_{oc}_{i}") for i in range(2)]
                for dc in range(DC):
                    for sc in range(2):
                        nc.tensor.matmul(
                            pss[sc],
                            lhsT=_r(wq[:, dc, oc * 128:(oc + 1) * 128]),
                            rhs=_r(xq[:, dc, sc * 512:(sc + 1) * 512]),
                            start=(dc == 0),
                            stop=(dc == DC - 1),
                        )
                for sc in range(2):
                    ev = p0ev.tile([128, 512], F32R, tag="ev")
                    nc.vector.tensor_scalar_mul(ev, pss[sc], SCALE)
                    nc.sync.dma_start(
                        out=qTs_r[:, oc, sc * 512:(sc + 1) * 512], in_=ev
                    )

        # ---------------- persistent tiles for P1..P3 ----------------
        with tc.tile_pool(name="persist", bufs=1) as per:
            kt = per.tile([128, DC, S], F32R)       # kT: [d_out, 2048]
            vv = per.tile([128, NKB, D], F32R)      # v:  [2048, d_out]
            ones_f = per.tile([128, 2], F32)
            ones = per.tile([128, 2], F32R)
            maskt = per.tile([128, 4, 256], F32R)
            nc.vector.memset(ones_f, 1.0)
            nc.vector.tensor_copy(ones, ones_f)
            nc.sync.dma_start(out=maskt, in_=masks[:].rearrange("m p c -> p m c"))

            # ---------------- P1: K projection -> kt ----------------
            with (
                tc.tile_pool(name="p1w", bufs=1) as p1w,
                tc.tile_pool(name="p1x", bufs=2) as p1x,
                tc.tile_pool(name="ps1", bufs=4, space="PSUM") as ps1,
            ):
                wk = p1w.tile([128, DC, D], F32R)
                nc.sync.dma_start(out=wk, in_=WkT_r)
                for sc in range(4):
                    xk = p1x.tile([128, DC, 512], F32R, tag="xs")
                    nc.sync.dma_start(
                        out=xk, in_=xT_r[:, :, sc * 512:(sc + 1) * 512]
                    )
                    for oc in range(8):
                        ps = ps1.tile([128, 512], F32, tag="ps1")
                        for dc in range(DC):
                            nc.tensor.matmul(
                                ps,
                                lhsT=_r(wk[:, dc, oc * 128:(oc + 1) * 128]),
                                rhs=_r(xk[:, dc, :]),
                                start=(dc == 0),
                                stop=(dc == DC - 1),
                            )
                        nc.vector.tensor_copy(
                            kt[:, oc, sc * 512:(sc + 1) * 512], ps
                        )

            # ---------------- P2: V projection -> vv ----------------
            with (
                tc.tile_pool(name="p2w", bufs=1) as p2w,
                tc.tile_pool(name="p2x", bufs=2) as p2x,
                tc.tile_pool(name="ps2", bufs=4, space="PSUM") as ps2,
            ):
                wv = p2w.tile([128, DC, D], F32R)
                nc.sync.dma_start(out=wv, in_=WvT_r)
                for sc in range(4):
                    xv = p2x.tile([128, DC, 512], F32R, tag="xs2")
                    nc.sync.dma_start(
                        out=xv, in_=xT_r[:, :, sc * 512:(sc + 1) * 512]
                    )
                    for sb in range(4):
                        pss = [ps2.tile([128, 512], F32, tag="ps2", name=f"ps2_{sc}_{sb}_{i}") for i in range(2)]
                        for dc in range(DC):
                            for oh in range(2):
                                nc.tensor.matmul(
                                    pss[oh],
                                    lhsT=_r(xv[:, dc, sb * 128:(sb + 1) * 128]),
                                    rhs=_r(wv[:, dc, oh * 512:(oh + 1) * 512]),
                                    start=(dc == 0),
                                    stop=(dc == DC - 1),
                                )
                        for oh in range(2):
                            nc.vector.tensor_copy(
                                vv[:, sc * 4 + sb, oh * 512:(oh + 1) * 512], pss[oh]
                            )

            # ---------------- P3: attention ----------------
            with (
                tc.tile_pool(name="qg", bufs=2) as qgp,
                tc.tile_pool(name="pt", bufs=19) as ptp,
                tc.tile_pool(name="ot", bufs=2) as otp,
                tc.tile_pool(name="sml", bufs=4) as smlp,
                tc.tile_pool(name="psT", bufs=2, space="PSUM") as psTp,
                tc.tile_pool(name="psav", bufs=3, space="PSUM") as psavp,
                tc.tile_pool(name="psl", bufs=2, space="PSUM") as pslp,
            ):
                for t in range(4):
                    L = 4 * t + 4
                    qg = qgp.tile([128, DC, 256], F32R, tag="qg")
                    nc.sync.dma_start(
                        out=qg, in_=qTs_r[:, :, t * 256:(t + 1) * 256]
                    )
                    pts = []
                    for kb in range(L):
                        ps = psTp.tile([128, 256], F32, tag="psT")
                        for dc in range(DC):
                            nc.tensor.matmul(
                                ps,
                                lhsT=_r(kt[:, dc, kb * 128:(kb + 1) * 128]),
                                rhs=_r(qg[:, dc, :]),
                                start=(dc == 0),
                                stop=(dc == DC - 1),
                            )
                        pt = ptp.tile([128, 256], F32R, tag="pt")
                        nc.scalar.activation(pt, ps, EXP)
                        kbr = kb - 4 * t
                        if kbr >= 0:
                            nc.vector.tensor_mul(pt, pt, maskt[:, kbr, :])
                        pts.append(pt)
                    for j in range(2):
                        qsl = slice(j * 128, (j + 1) * 128)
                        lps = pslp.tile([128, 2], F32, tag="psl")
                        for kb in range(L):
                            nc.tensor.matmul(
                                lps,
                                lhsT=_r(pts[kb][:, qsl]),
                                rhs=_r(ones),
                                start=(kb == 0),
                                stop=(kb == L - 1),
                            )
                        rec = smlp.tile([128, 1], F32, tag="rec")
                        nc.vector.reciprocal(rec, lps[:, 0:1])
                        ot = otp.tile([128, D], F32, tag="ot")
                        for oh in range(2):
                            avp = psavp.tile([128, 512], F32, tag="psav")
                            for kb in range(L):
                                nc.tensor.matmul(
                                    avp,
                                    lhsT=_r(pts[kb][:, qsl]),
                                    rhs=_r(vv[:, kb, oh * 512:(oh + 1) * 512]),
                                    start=(kb == 0),
                                    stop=(kb == L - 1),
                                )
                            nc.vector.tensor_scalar_mul(
                                ot[:, oh * 512:(oh + 1) * 512], avp, rec
                            )
                        nc.sync.dma_start(
                            out=out[t * 256 + j * 128: t * 256 + (j + 1) * 128, :],
                            in_=ot,
                        )
    nc.compile()
    return nc


def _query_cols(sub):
    return np.concatenate(
        [
            np.arange((4 * t + 2 * sub) * 128, (4 * t + 2 * sub + 2) * 128)
            for t in range(4)
        ]
    )


def _masks(sub):
    m = np.zeros((4, 128, 256), np.float32)
    p = np.arange(128)[:, None]
    j = np.arange(256)[None, :]
    qoff = (2 * sub + j // 128) * 128 + j % 128
    for kbr in range(4):
        m[kbr] = (kbr * 128 + p <= qoff).astype(np.float32)
    return m


def kernel(x, Wq, Wk, Wv, _trace=False):
    if "nc" not in _cache:
        _cache["nc"] = build_nc()
    nc = _cache["nc"]

    x = np.ascontiguousarray(np.asarray(x, dtype=np.float32))
    WqT = np.ascontiguousarray(np.asarray(Wq, np.float32).T)
    WkT = np.ascontiguousarray(np.asarray(Wk, np.float32).T)
    WvT = np.ascontiguousarray(np.asarray(Wv, np.float32).T)

    in_maps = []
    for c in range(8):
        b, sub = c // 2, c % 2
        xT = np.ascontiguousarray(x[b].T)
        in_maps.append(
            {
                "xT": xT,
                "xTq": np.ascontiguousarray(xT[:, _query_cols(sub)]),
                "WqT": WqT,
                "WkT": WkT,
                "WvT": WvT,
                "masks": _masks(sub),
            }
        )

    res = run_bass_kernel_spmd(
        nc, in_maps, core_ids=list(range(8)), trace=_trace
    )
    full = np.empty((B, S, D), np.float32)
    for c in range(8):
        b, sub = c // 2, c % 2
        full[b, _query_cols(sub)] = res.results[c]["out"]
    if _trace:
        _cache["last_result"] = res
    return full
